# revision 28
# baseline (speedup 1.0000x reference)
"""Differential multi-head attention on 8 TRN2 NeuronCores.

Sharding: core c handles batch b = c//2 and head-half hh = c%2
(4 of 8 effective heads = 8 of 16 raw heads). Host sums the two
per-batch partial out-projections (the "all-reduce") and reassembles
(L, N, D) fp32.

Per-core scheme (v2, transposed scores):
  scoresT[m, l] = k_m . q_l computed directly on the PE (stationary =
  k chunk, moving = q), so exp(scoresT) tiles are exactly the
  stationary operand attnV needs -- no SBUF transposes of the
  probability matrix and no full-matrix lambda-combine on DVE.
  Softmax row sums ride along as a ones-column appended to V: each
  attnV matmul outputs 129 cols = [attn@V | rowsum]. The differential
  combine o1/s1 - lam*o2/s2 and headwise RMSNorm run on small
  [128,128] tiles. Unit u's scores+exp (ACT-bound) overlap unit u-1's
  attnV matmuls (PE), with q/k/v projections filling remaining PE
  slack. rsqrt for RMSNorm is exp(-0.5*ln(x)) to stay in the one ACT
  table set (natural_log_exp).

All matmuls in fp16 with fp32 PSUM accumulation.
"""
import os

import numpy as np

import concourse.bass as bass
import concourse.mybir as mybir
import concourse.tile as tile
from concourse import bass_utils


def _enable_fwl():
    """Re-enable the compiler's fast-weight-load path (off by default in
    this environment); stationary-operand loads then run at 2 cols/cycle."""
    try:
        from concourse import compiler_utils

        flags = compiler_utils.get_compiler_flags()
        new = [f.replace("--enable-ldw-opt=false", "--enable-ldw-opt=true")
               for f in flags]
        if new != flags:
            compiler_utils.set_compiler_flags(new)
    except Exception:
        pass


if os.environ.get("BASS_FWL", "0") == "1":
    _enable_fwl()

L = 1024          # sequence length
B = 4             # batch
D = 1024          # embed dim
P = 128           # partitions
HD = 64           # head dim
HEFF = 4          # effective heads per core (of 8 total)
DH2 = 2 * HD      # 128, v head dim / rmsnorm width
KO = D // P       # 8 contraction chunks
NMT = L // P      # 8 m-chunks (key/value tiles)
NLT = L // P      # 8 l-tiles (query tiles)
LAMBDA_INIT = 0.8
EPS = 1e-5
SCALING = HD ** -0.5
VW = 132          # vt row width: 128 v cols + ones col + pad

F32 = mybir.dt.float32
F16 = mybir.dt.float16
AF = mybir.ActivationFunctionType
ALU = mybir.AluOpType

# ---------------------------------------------------------------------------
# wait-budget post-pass (TRN2 ISA instructions carry a single wait slot;
# excess waits move to InstNoOp on the same engine stream)
_WAIT_EXEMPT = {
    "InstEventSemaphore", "InstRegisterMove", "InstUnconditionalBranch",
    "InstCall", "InstHalt", "InstNoOp", "InstAllEngineBarrier",
    "InstBranchHint", "InstCompareAndBranch", "InstFusedRegOps",
    "InstRegisterAlu",
}
_waitfix_counter = [0]


def _split_waits(nc):
    n_split = 0
    for f in nc.m.functions:
        for bb in f.blocks:
            il = bb.instructions
            out = []
            changed = False
            for inst in il:
                tn = type(inst).__name__
                si = inst.sync_info
                waits = list(si.on_wait) if si is not None and si.on_wait else []
                if tn in _WAIT_EXEMPT or len(waits) <= 1:
                    out.append(inst)
                    continue
                excess, keep = waits[:-1], waits[-1:]
                movable = [w for w in excess if w.wait_reg is None]
                stuck = [w for w in excess if w.wait_reg is not None]
                for w in movable:
                    _waitfix_counter[0] += 1
                    out.append(mybir.InstNoOp(
                        name=f"I-waitnop-{_waitfix_counter[0]}",
                        engine=inst.engine, ins=[], outs=[],
                        sync_info=mybir.SyncInfo(on_wait=[w], on_update=[]),
                    ))
                    n_split += 1
                si.on_wait = stuck + keep
                changed = True
                out.append(inst)
            if changed:
                bb.instructions = out
    return n_split


# psum slot offsets for the attnV accumulators: 8 slots of 129 fp32 at a
# 132-word (528B, 16B-aligned) stride, packed 3-3-2 into 3 banks (512 fp32
# each) so no slot crosses a bank
def _slot_off(i):
    return (i // 3) * 512 + (i % 3) * 132


# ---------------------------------------------------------------------------

def build_nc(split_waits=True):
    nc = bass.Bass("TRN2", target_bir_lowering=False, debug=False)

    xt_d = nc.dram_tensor("xt", [D, L], F16, kind="ExternalInput").ap()
    wq_d = nc.dram_tensor("wq", [D, HEFF * DH2], F16, kind="ExternalInput").ap()
    wk_d = nc.dram_tensor("wk", [D, HEFF * DH2], F16, kind="ExternalInput").ap()
    wv_d = nc.dram_tensor("wv", [D, HEFF * DH2], F16, kind="ExternalInput").ap()
    wo_d = nc.dram_tensor("wo", [HEFF * DH2, D], F16, kind="ExternalInput").ap()
    lam_d = nc.dram_tensor("lamneg", [P, 1], F32, kind="ExternalInput").ap()
    ones_d = nc.dram_tensor("ones", [P, NMT, HEFF, 1], F16,
                            kind="ExternalInput").ap()
    ident_d = nc.dram_tensor("ident", [P, P], F16, kind="ExternalInput").ap()
    out_d = nc.dram_tensor("out", [L, D], F32, kind="ExternalOutput").ap()

    with tile.TileContext(nc) as tc:
        with (
            tc.tile_pool(name="weights", bufs=1) as wpool,
            tc.tile_pool(name="proj", bufs=1) as projpool,
            tc.tile_pool(name="stats", bufs=1) as spool,
            tc.tile_pool(name="attn", bufs=1) as apool,
        ):
            # ---------------- loads ----------------
            lamneg = wpool.tile([P, 1], F32)
            nc.gpsimd.dma_start(lamneg[:], lam_d[:])
            xt_t = wpool.tile([P, KO, L], F16)
            xt_r = xt_d.rearrange("(ko p) l -> p ko l", p=P)
            wq_t = wpool.tile([P, KO, 512], F16)
            wq_r = wq_d.rearrange("(ko p) n -> p ko n", p=P)
            wk_t = wpool.tile([P, KO, 512], F16)
            wk_r = wk_d.rearrange("(ko p) n -> p ko n", p=P)
            wv_t = wpool.tile([P, KO, 512], F16)
            wv_r = wv_d.rearrange("(ko p) n -> p ko n", p=P)
            # three trigger queues in parallel: sync=xt+wv, scalar=wq (ACT
            # idle this early), gpsimd=wk+wo
            for ko in range(KO):
                nc.sync.dma_start(xt_t[:, ko], xt_r[:, ko])
                nc.scalar.dma_start(wq_t[:, ko], wq_r[:, ko])
                nc.gpsimd.dma_start(wk_t[:, ko], wk_r[:, ko])
            for ko in range(KO):
                nc.sync.dma_start(wv_t[:, ko], wv_r[:, ko])
            wo_t = wpool.tile([P, HEFF, D], F16)
            nc.gpsimd.dma_start(wo_t[:], wo_d.rearrange("(u p) n -> p u n", p=P))

            act_scr = spool.tile([P, 4], F32)
            dve_scr = spool.tile([P, 4], F32)
            # init touches: load the exp/ln ACT table set early (overlaps
            # the input DMAs) and let DVE observe the consts load
            nc.vector.tensor_copy(dve_scr[0:1, 0:1], lamneg[0:1, 0:1])
            nc.scalar.activation(act_scr[0:1, 0:1], lamneg[0:1, 0:1], AF.Exp)
            nc.scalar.activation(act_scr[0:1, 1:2], act_scr[0:1, 0:1], AF.Ln)

            # ---------------- persistent SBUF tensors ----------------
            qt = projpool.tile([P, HEFF, L], F16)    # (dh%128, u, l)
            kt = projpool.tile([P, HEFF, L], F16)
            vt = projpool.tile([P, NMT, HEFF, VW], F16)  # (m%128, mt, u, d|1)
            nc.gpsimd.dma_start(vt[:, :, :, DH2:DH2 + 1], ones_d[:])
            ident = projpool.tile([P, P], F16)
            nc.gpsimd.dma_start(ident[:], ident_d[:])

            # exp(scoresT) double buffer: [m%128, h, mt, l] per unit parity
            expT = [apool.tile([P, 2, NMT, L], F16, name=f"expT{i}")
                    for i in range(2)]
            attn2 = apool.tile([P, NLT, HEFF, DH2], F16)  # (l%128, lt, u, d)
            attnT = apool.tile([P, NLT, HEFF, P], F16)    # (d, lt, u, l%128)

            rec1 = [spool.tile([P, NLT], F32, name=f"rec1_{u}") for u in range(HEFF)]
            rec2 = [spool.tile([P, NLT], F32, name=f"rec2_{u}") for u in range(HEFF)]
            ss_t = [spool.tile([P, NLT], F32, name=f"ss_{u}") for u in range(HEFF)]
            den_t = [spool.tile([P, NLT], F32, name=f"den_{u}") for u in range(HEFF)]
            lnd_t = [spool.tile([P, NLT], F32, name=f"lnd_{u}") for u in range(HEFF)]
            rsc_t = [spool.tile([P, NLT], F32, name=f"rsc_{u}") for u in range(HEFF)]
            sqjunk = spool.tile([P, DH2], F16)

            with (
                tc.tile_pool(name="ps_proj", bufs=1, space="PSUM") as ps_proj,
                tc.tile_pool(name="ps_att", bufs=1, space="PSUM") as ps_att,
                tc.tile_pool(name="diffs", bufs=10) as dpool,
                tc.tile_pool(name="t2s", bufs=4) as tpool,
            ):
                att_ps = ps_att.tile([P, 3 * 512], F32)  # 3 banks, 8 slots

                # ---------- projection group emitters ----------
                def emit_qk_group(u, w_t, outt, nt, isq):
                    ps = ps_proj.tile([P, 512], F32, tag="pp")
                    for ko in range(KO):
                        nc.tensor.matmul(
                            ps[:],
                            w_t[:, ko, u * P:(u + 1) * P],
                            xt_t[:, ko, nt * 512:(nt + 1) * 512],
                            start=(ko == 0), stop=(ko == KO - 1),
                        )
                    dst = outt[:, u, nt * 512:(nt + 1) * 512]
                    if isq:
                        nc.vector.tensor_scalar_mul(dst, ps[:], SCALING)
                    else:
                        nc.vector.tensor_copy(dst, ps[:])

                def emit_v_group(mt):
                    ps = ps_proj.tile([P, 512], F32, tag="pp")
                    for ko in range(KO):
                        nc.tensor.matmul(
                            ps[:],
                            xt_t[:, ko, mt * P:(mt + 1) * P],
                            wv_t[:, ko, :],
                            start=(ko == 0), stop=(ko == KO - 1),
                        )
                    nc.vector.tensor_copy(
                        vt[:, mt, :, 0:DH2],
                        ps[:].rearrange("p (u d) -> p u d", u=HEFF),
                    )

                # proj work queue consumed between score slots: v groups
                # first (attnV(0) needs all of v at unit-1 start), then
                # q before k per unit (scores need q fully, k per-chunk)
                projq = []
                for mt in range(3, NMT):
                    projq.append(lambda mt=mt: emit_v_group(mt))
                for u in range(1, HEFF):
                    for w_t, outt, isq in ((wq_t, qt, True), (wk_t, kt, False)):
                        for nt in range(2):
                            projq.append(
                                lambda u=u, w=w_t, o=outt, nt=nt, q=isq:
                                    emit_qk_group(u, w, o, nt, q))
                # per unit-slot schedule: how many proj groups to pop
                # (u=0: ~1.5/slot; u=1: 1/slot until drained)
                projq.reverse()  # pop from end

                # ---------- attnV + combine emitters ----------
                diffs = [[None] * NLT for _ in range(HEFF)]

                def emit_attnv_lt(u, lt):
                    """attnV for (u, lt), both heads; 16 MMs + combine."""
                    par = u % 2
                    for h in range(2):
                        off = _slot_off(2 * (lt % 4) + h)
                        pav = att_ps[:, off:off + 129]
                        for mt in range(NMT):
                            nc.tensor.matmul(
                                pav,
                                expT[par][:, h, mt, lt * P:(lt + 1) * P],
                                vt[:, mt, u, 0:DH2 + 1],
                                start=(mt == 0), stop=(mt == NMT - 1),
                            )

                def emit_combine_lt(u, lt):
                    off1 = _slot_off(2 * (lt % 4) + 0)
                    off2 = _slot_off(2 * (lt % 4) + 1)
                    o1 = att_ps[:, off1:off1 + DH2]
                    s1 = att_ps[:, off1 + DH2:off1 + DH2 + 1]
                    o2 = att_ps[:, off2:off2 + DH2]
                    s2 = att_ps[:, off2 + DH2:off2 + DH2 + 1]
                    nc.vector.reciprocal(rec1[u][:, lt:lt + 1], s1)
                    nc.vector.reciprocal(rec2[u][:, lt:lt + 1], s2)
                    # t2 = o2 * (1/s2) * (-lam);  diff = o1*(1/s1) + t2
                    t2 = tpool.tile([P, DH2], F32, tag="t2")
                    nc.vector.tensor_scalar(
                        t2[:], o2, rec2[u][:, lt:lt + 1], lamneg[:],
                        op0=ALU.mult, op1=ALU.mult,
                    )
                    diff = dpool.tile([P, DH2], F16, tag="diff")
                    nc.vector.scalar_tensor_tensor(
                        diff[:], o1, rec1[u][:, lt:lt + 1], t2[:],
                        op0=ALU.mult, op1=ALU.add,
                    )
                    diffs[u][lt] = diff
                    nc.vector.scalar_tensor_tensor(
                        sqjunk[:], diff[:], 1.0, diff[:],
                        op0=ALU.mult, op1=ALU.mult,
                        accum_out=ss_t[u][:, lt:lt + 1],
                    )

                def emit_stats(u, lo, hi):
                    """rsqrt(mean(diff^2)+eps) for lt in [lo, hi) and the
                    final attn2 scale (* rms * (1-lambda_init))."""
                    sl = slice(lo, hi)
                    nc.vector.tensor_scalar(
                        den_t[u][:, sl], ss_t[u][:, sl], 1.0 / DH2, EPS,
                        op0=ALU.mult, op1=ALU.add,
                    )
                    nc.scalar.activation(lnd_t[u][:, sl], den_t[u][:, sl], AF.Ln)
                    nc.scalar.activation(
                        rsc_t[u][:, sl], lnd_t[u][:, sl], AF.Exp, scale=-0.5)
                    for lt in range(lo, hi):
                        nc.vector.tensor_scalar(
                            attn2[:, lt, u, :], diffs[u][lt][:],
                            rsc_t[u][:, lt:lt + 1], 1.0 - LAMBDA_INIT,
                            op0=ALU.mult, op1=ALU.mult,
                        )

                # ---------- lead-in: unit-0 q/k projection + early v ----------
                # order q-nt0, k-nt0 first so the first score MMs (which
                # need only the nt0 halves) can issue as early as possible
                for nt in range(2):
                    for w_t, outt, isq in ((wq_t, qt, True), (wk_t, kt, False)):
                        emit_qk_group(0, w_t, outt, nt, isq)
                for mt in range(3):
                    emit_v_group(mt)

                # ---------- main pipeline ----------
                with tc.tile_pool(name="ps_s", bufs=2, space="PSUM") as ps_s:
                    for u in range(HEFF):
                        for mt in range(NMT):
                            # scoresT MMs: stationary k chunk, moving q;
                            # the two heads run in PE row groups 0-63/64-127
                            # (interleaved h0/h1 so the pairs overlap)
                            pss = [ps_s.tile([P, L], F32, tag="scores",
                                             name=f"ps_{u}_{mt}_{h}")
                                   for h in range(2)]
                            for nl in range(2):
                                for h in range(2):
                                    base = h * HD
                                    nc.tensor.matmul(
                                        pss[h][:, nl * 512:(nl + 1) * 512],
                                        kt[base:base + HD, u, mt * P:(mt + 1) * P],
                                        qt[base:base + HD, u, nl * 512:(nl + 1) * 512],
                                        start=True, stop=True,
                                    )
                            for h in range(2):
                                nc.scalar.activation(
                                    expT[u % 2][:, h, mt, :], pss[h][:], AF.Exp,
                                )
                            # prev unit's attnV rides this slot on the PE
                            if u >= 1:
                                emit_attnv_lt(u - 1, mt)
                                emit_combine_lt(u - 1, mt)
                                if mt == 3:
                                    emit_stats(u - 1, 0, 4)
                                if mt == 7:
                                    emit_stats(u - 1, 4, 8)
                            # remaining projections fill PE slack
                            if projq:
                                projq.pop()()
                        # end mt loop
                    # end unit loop

                # ---------- tail: attnV(3) + PE transposes + outproj ----------
                with (
                    tc.tile_pool(name="ps_t", bufs=2, space="PSUM") as ps_t,
                    tc.tile_pool(name="ps_o", bufs=2, space="PSUM") as ps_o,
                    tc.tile_pool(name="outsb", bufs=6) as outsb,
                ):
                    u = HEFF - 1

                    def emit_transpose_lt(lt):
                        for uu in range(HEFF):
                            pt = ps_t.tile([P, P], F16, tag="pt")
                            nc.tensor.transpose(
                                pt[:], attn2[:, lt, uu, :], ident[:])
                            if uu % 2 == 0:
                                nc.vector.tensor_copy(attnT[:, lt, uu, :], pt[:])
                            else:
                                nc.scalar.copy(attnT[:, lt, uu, :], pt[:])

                    def emit_outproj_lt(lt):
                        for nt in range(2):
                            ps = ps_o.tile([P, 512], F32, tag="po")
                            for uu in range(HEFF):
                                nc.tensor.matmul(
                                    ps[:],
                                    attnT[:, lt, uu, :],
                                    wo_t[:, uu, nt * 512:(nt + 1) * 512],
                                    start=(uu == 0), stop=(uu == HEFF - 1),
                                )
                            osb = outsb.tile([P, 512], F32, tag="osb")
                            if (lt * 2 + nt) % 2 == 0:
                                nc.vector.tensor_copy(osb[:], ps[:])
                            else:
                                nc.scalar.copy(osb[:], ps[:])
                            nc.gpsimd.dma_start(
                                out_d[lt * P:(lt + 1) * P, nt * 512:(nt + 1) * 512],
                                osb[:],
                            )

                    # per lt-pair: attnV + combine + stats, then transpose
                    # and outproj of the PREVIOUS pair ride the PE between
                    # this pair's matmuls
                    for q in range(4):
                        for lt in (2 * q, 2 * q + 1):
                            emit_attnv_lt(u, lt)
                            emit_combine_lt(u, lt)
                        if q >= 1:
                            emit_transpose_lt(2 * q - 2)
                            emit_transpose_lt(2 * q - 1)
                        emit_stats(u, 2 * q, 2 * q + 2)
                        if q >= 1:
                            emit_outproj_lt(2 * q - 2)
                            emit_outproj_lt(2 * q - 1)
                    for lt2 in (6, 7):
                        emit_transpose_lt(lt2)
                    for lt2 in (6, 7):
                        emit_outproj_lt(lt2)

    if split_waits:
        _split_waits(nc)
    return nc


_NC_CACHE = None


def _get_nc():
    global _NC_CACHE
    if _NC_CACHE is None:
        _NC_CACHE = build_nc(split_waits=(__import__("os").environ.get("NO_SPLIT_WAITS","0")!="1"))
    return _NC_CACHE


def kernel(**inputs):
    nc = _get_nc()
    in_maps = _make_in_maps(inputs)
    res = bass_utils.run_bass_kernel_spmd(nc, in_maps, core_ids=list(range(8)))

    out = np.empty((L, B, D), dtype=np.float32)
    for b in range(B):
        out[:, b, :] = res.results[2 * b]["out"] + res.results[2 * b + 1]["out"]
    return out


def _make_in_maps(inputs):
    query = np.asarray(inputs["query"], dtype=np.float32)
    Wq = np.asarray(inputs["Wq"], dtype=np.float32)
    Wk = np.asarray(inputs["Wk"], dtype=np.float32)
    Wv = np.asarray(inputs["Wv"], dtype=np.float32)
    Wo = np.asarray(inputs["Wo"], dtype=np.float32)
    lq1 = np.asarray(inputs["lq1"], dtype=np.float64)
    lk1 = np.asarray(inputs["lk1"], dtype=np.float64)
    lq2 = np.asarray(inputs["lq2"], dtype=np.float64)
    lk2 = np.asarray(inputs["lk2"], dtype=np.float64)
    lam = float(np.exp(np.sum(lq1 * lk1)) - np.exp(np.sum(lq2 * lk2)) + LAMBDA_INIT)
    lamneg = np.full((P, 1), -lam, dtype=np.float32)
    in_maps = []
    for c in range(8):
        b, hh = c // 2, c % 2
        sl = slice(hh * 512, (hh + 1) * 512)
        in_maps.append({
            "xt": np.ascontiguousarray(query[:, b, :].T.astype(np.float16)),
            "wq": np.ascontiguousarray(Wq[:, sl].astype(np.float16)),
            "wk": np.ascontiguousarray(Wk[:, sl].astype(np.float16)),
            "wv": np.ascontiguousarray(Wv[:, sl].astype(np.float16)),
            "wo": np.ascontiguousarray(Wo[sl, :].astype(np.float16)),
            "lamneg": lamneg,
            "ones": np.ones((P, NMT, HEFF, 1), dtype=np.float16),
            "ident": np.eye(P, dtype=np.float16),
        })
    return in_maps


def kernel_traced(**inputs):
    """Run with NTFF tracing; returns max-core exec time in ns (or None)."""
    nc = _get_nc()
    res = bass_utils.run_bass_kernel_spmd(
        nc, _make_in_maps(inputs), core_ids=list(range(8)), trace=True,
    )
    if res.instructions_and_trace is not None:
        print("trace:", res.instructions_and_trace[1])
    print("per-core mean exec:", res.mean_exec_time_ns,
          "max core:", res.max_exec_time_core_id)
    return res.exec_time_ns


# revision 30
# speedup vs baseline: 1.0273x; 1.0273x over previous
"""Differential multi-head attention on 8 TRN2 NeuronCores.

Sharding: core c handles batch b = c//2 and head-half hh = c%2
(4 of 8 effective heads = 8 of 16 raw heads). Host sums the two
per-batch partial out-projections (the "all-reduce") and reassembles
(L, N, D) fp32.

Per-core scheme (v2, transposed scores):
  scoresT[m, l] = k_m . q_l computed directly on the PE (stationary =
  k chunk, moving = q), so exp(scoresT) tiles are exactly the
  stationary operand attnV needs -- no SBUF transposes of the
  probability matrix and no full-matrix lambda-combine on DVE.
  Softmax row sums ride along as a ones-column appended to V: each
  attnV matmul outputs 129 cols = [attn@V | rowsum]. The differential
  combine o1/s1 - lam*o2/s2 and headwise RMSNorm run on small
  [128,128] tiles. Unit u's scores+exp (ACT-bound) overlap unit u-1's
  attnV matmuls (PE), with q/k/v projections filling remaining PE
  slack. rsqrt for RMSNorm is exp(-0.5*ln(x)) to stay in the one ACT
  table set (natural_log_exp).

All matmuls in fp16 with fp32 PSUM accumulation.
"""
import os

import numpy as np

import concourse.bass as bass
import concourse.mybir as mybir
import concourse.tile as tile
from concourse import bass_utils


def _enable_fwl():
    """Re-enable the compiler's fast-weight-load path (off by default in
    this environment); stationary-operand loads then run at 2 cols/cycle."""
    try:
        from concourse import compiler_utils

        flags = compiler_utils.get_compiler_flags()
        new = [f.replace("--enable-ldw-opt=false", "--enable-ldw-opt=true")
               for f in flags]
        if new != flags:
            compiler_utils.set_compiler_flags(new)
    except Exception:
        pass


if os.environ.get("BASS_FWL", "0") == "1":
    _enable_fwl()

L = 1024          # sequence length
B = 4             # batch
D = 1024          # embed dim
P = 128           # partitions
HD = 64           # head dim
HEFF = 4          # effective heads per core (of 8 total)
DH2 = 2 * HD      # 128, v head dim / rmsnorm width
KO = D // P       # 8 contraction chunks
NMT = L // P      # 8 m-chunks (key/value tiles)
NLT = L // P      # 8 l-tiles (query tiles)
LAMBDA_INIT = 0.8
EPS = 1e-5
SCALING = HD ** -0.5
VW = 132          # vt row width: 128 v cols + ones col + pad

F32 = mybir.dt.float32
F16 = mybir.dt.float16
AF = mybir.ActivationFunctionType
ALU = mybir.AluOpType

# ---------------------------------------------------------------------------
# wait-budget post-pass (TRN2 ISA instructions carry a single wait slot;
# excess waits move to InstNoOp on the same engine stream)
_WAIT_EXEMPT = {
    "InstEventSemaphore", "InstRegisterMove", "InstUnconditionalBranch",
    "InstCall", "InstHalt", "InstNoOp", "InstAllEngineBarrier",
    "InstBranchHint", "InstCompareAndBranch", "InstFusedRegOps",
    "InstRegisterAlu",
}
_waitfix_counter = [0]


def _split_waits(nc):
    n_split = 0
    for f in nc.m.functions:
        for bb in f.blocks:
            il = bb.instructions
            out = []
            changed = False
            for inst in il:
                tn = type(inst).__name__
                si = inst.sync_info
                waits = list(si.on_wait) if si is not None and si.on_wait else []
                if tn in _WAIT_EXEMPT or len(waits) <= 1:
                    out.append(inst)
                    continue
                excess, keep = waits[:-1], waits[-1:]
                movable = [w for w in excess if w.wait_reg is None]
                stuck = [w for w in excess if w.wait_reg is not None]
                for w in movable:
                    _waitfix_counter[0] += 1
                    out.append(mybir.InstNoOp(
                        name=f"I-waitnop-{_waitfix_counter[0]}",
                        engine=inst.engine, ins=[], outs=[],
                        sync_info=mybir.SyncInfo(on_wait=[w], on_update=[]),
                    ))
                    n_split += 1
                si.on_wait = stuck + keep
                changed = True
                out.append(inst)
            if changed:
                bb.instructions = out
    return n_split


# psum slot offsets for the attnV accumulators: 8 slots of 129 fp32 at a
# 132-word (528B, 16B-aligned) stride, packed 3-3-2 into 3 banks (512 fp32
# each) so no slot crosses a bank
def _slot_off(i):
    return (i // 3) * 512 + (i % 3) * 132


# ---------------------------------------------------------------------------

def build_nc(split_waits=True):
    nc = bass.Bass("TRN2", target_bir_lowering=False, debug=False)

    xt_d = nc.dram_tensor("xt", [D, L], F16, kind="ExternalInput").ap()
    wq_d = nc.dram_tensor("wq", [D, HEFF * DH2], F16, kind="ExternalInput").ap()
    wk_d = nc.dram_tensor("wk", [D, HEFF * DH2], F16, kind="ExternalInput").ap()
    wv_d = nc.dram_tensor("wv", [D, HEFF * DH2], F16, kind="ExternalInput").ap()
    wo_d = nc.dram_tensor("wo", [HEFF * DH2, D], F16, kind="ExternalInput").ap()
    lam_d = nc.dram_tensor("lamneg", [P, 1], F32, kind="ExternalInput").ap()
    ones_d = nc.dram_tensor("ones", [P, NMT, HEFF, 1], F16,
                            kind="ExternalInput").ap()
    ident_d = nc.dram_tensor("ident", [P, P], F16, kind="ExternalInput").ap()
    out_d = nc.dram_tensor("out", [L, D], F32, kind="ExternalOutput").ap()

    with tile.TileContext(nc) as tc:
        with (
            tc.tile_pool(name="weights", bufs=1) as wpool,
            tc.tile_pool(name="proj", bufs=1) as projpool,
            tc.tile_pool(name="stats", bufs=1) as spool,
            tc.tile_pool(name="attn", bufs=1) as apool,
        ):
            # ---------------- loads ----------------
            lamneg = wpool.tile([P, 1], F32)
            nc.gpsimd.dma_start(lamneg[:], lam_d[:])
            xt_t = wpool.tile([P, KO, L], F16)
            xt_r = xt_d.rearrange("(ko p) l -> p ko l", p=P)
            wq_t = wpool.tile([P, KO, 512], F16)
            wq_r = wq_d.rearrange("(ko p) n -> p ko n", p=P)
            wk_t = wpool.tile([P, KO, 512], F16)
            wk_r = wk_d.rearrange("(ko p) n -> p ko n", p=P)
            wv_t = wpool.tile([P, KO, 512], F16)
            wv_r = wv_d.rearrange("(ko p) n -> p ko n", p=P)
            # sync queue: first x/wq chunks then all of wv (so v-proj can
            # start during the lead); gpsimd queue: the rest
            nc.sync.dma_start(wq_t[:, 0], wq_r[:, 0])
            nc.sync.dma_start(xt_t[:, 0], xt_r[:, 0])
            for ko in range(1, KO):
                nc.gpsimd.dma_start(wq_t[:, ko], wq_r[:, ko])
                nc.gpsimd.dma_start(xt_t[:, ko], xt_r[:, ko])
            for ko in range(KO):
                nc.sync.dma_start(wv_t[:, ko], wv_r[:, ko])
                nc.gpsimd.dma_start(wk_t[:, ko], wk_r[:, ko])
            wo_t = wpool.tile([P, HEFF, D], F16)
            nc.gpsimd.dma_start(wo_t[:], wo_d.rearrange("(u p) n -> p u n", p=P))

            act_scr = spool.tile([P, 4], F32)
            dve_scr = spool.tile([P, 4], F32)
            # init touches: load the exp/ln ACT table set early (overlaps
            # the input DMAs) and let DVE observe the consts load
            nc.vector.tensor_copy(dve_scr[0:1, 0:1], lamneg[0:1, 0:1])
            nc.scalar.activation(act_scr[0:1, 0:1], lamneg[0:1, 0:1], AF.Exp)
            nc.scalar.activation(act_scr[0:1, 1:2], act_scr[0:1, 0:1], AF.Ln)

            # ---------------- persistent SBUF tensors ----------------
            qt = projpool.tile([P, HEFF, L], F16)    # (dh%128, u, l)
            kt = projpool.tile([P, HEFF, L], F16)
            vt = projpool.tile([P, NMT, HEFF, VW], F16)  # (m%128, mt, u, d|1)
            nc.gpsimd.dma_start(vt[:, :, :, DH2:DH2 + 1], ones_d[:])
            ident = projpool.tile([P, P], F16)
            nc.gpsimd.dma_start(ident[:], ident_d[:])

            # exp(scoresT) double buffer: [m%128, h, mt, l] per unit parity
            expT = [apool.tile([P, 2, NMT, L], F16, name=f"expT{i}")
                    for i in range(2)]
            attn2 = apool.tile([P, NLT, HEFF, DH2], F16)  # (l%128, lt, u, d)
            attnT = apool.tile([P, NLT, HEFF, P], F16)    # (d, lt, u, l%128)

            rec1 = [spool.tile([P, NLT], F32, name=f"rec1_{u}") for u in range(HEFF)]
            rec2 = [spool.tile([P, NLT], F32, name=f"rec2_{u}") for u in range(HEFF)]
            ss_t = [spool.tile([P, NLT], F32, name=f"ss_{u}") for u in range(HEFF)]
            den_t = [spool.tile([P, NLT], F32, name=f"den_{u}") for u in range(HEFF)]
            lnd_t = [spool.tile([P, NLT], F32, name=f"lnd_{u}") for u in range(HEFF)]
            rsc_t = [spool.tile([P, NLT], F32, name=f"rsc_{u}") for u in range(HEFF)]
            sqjunk = spool.tile([P, DH2], F16)

            with (
                tc.tile_pool(name="ps_proj", bufs=1, space="PSUM") as ps_proj,
                tc.tile_pool(name="ps_att", bufs=1, space="PSUM") as ps_att,
                tc.tile_pool(name="diffs", bufs=10) as dpool,
                tc.tile_pool(name="t2s", bufs=4) as tpool,
            ):
                att_ps = ps_att.tile([P, 3 * 512], F32)  # 3 banks, 8 slots

                # ---------- projection group emitters ----------
                def emit_qk_group(u, w_t, outt, nt, isq):
                    ps = ps_proj.tile([P, 512], F32, tag="pp")
                    for ko in range(KO):
                        nc.tensor.matmul(
                            ps[:],
                            w_t[:, ko, u * P:(u + 1) * P],
                            xt_t[:, ko, nt * 512:(nt + 1) * 512],
                            start=(ko == 0), stop=(ko == KO - 1),
                        )
                    dst = outt[:, u, nt * 512:(nt + 1) * 512]
                    if isq:
                        nc.vector.tensor_scalar_mul(dst, ps[:], SCALING)
                    else:
                        nc.vector.tensor_copy(dst, ps[:])

                def emit_v_group(mt):
                    ps = ps_proj.tile([P, 512], F32, tag="pp")
                    for ko in range(KO):
                        nc.tensor.matmul(
                            ps[:],
                            xt_t[:, ko, mt * P:(mt + 1) * P],
                            wv_t[:, ko, :],
                            start=(ko == 0), stop=(ko == KO - 1),
                        )
                    nc.vector.tensor_copy(
                        vt[:, mt, :, 0:DH2],
                        ps[:].rearrange("p (u d) -> p u d", u=HEFF),
                    )

                # proj work queue consumed between score slots: v groups
                # first (attnV(0) needs all of v at unit-1 start), then
                # q before k per unit (scores need q fully, k per-chunk)
                projq = []
                for mt in range(3, NMT):
                    projq.append(lambda mt=mt: emit_v_group(mt))
                for u in range(1, HEFF):
                    for w_t, outt, isq in ((wq_t, qt, True), (wk_t, kt, False)):
                        for nt in range(2):
                            projq.append(
                                lambda u=u, w=w_t, o=outt, nt=nt, q=isq:
                                    emit_qk_group(u, w, o, nt, q))
                # per unit-slot schedule: how many proj groups to pop
                # (u=0: ~1.5/slot; u=1: 1/slot until drained)
                projq.reverse()  # pop from end

                # ---------- attnV + combine emitters ----------
                diffs = [[None] * NLT for _ in range(HEFF)]

                def emit_attnv_lt(u, lt):
                    """attnV for (u, lt), both heads; 16 MMs + combine."""
                    par = u % 2
                    for h in range(2):
                        off = _slot_off(2 * (lt % 4) + h)
                        pav = att_ps[:, off:off + 129]
                        for mt in range(NMT):
                            nc.tensor.matmul(
                                pav,
                                expT[par][:, h, mt, lt * P:(lt + 1) * P],
                                vt[:, mt, u, 0:DH2 + 1],
                                start=(mt == 0), stop=(mt == NMT - 1),
                            )

                def emit_combine_lt(u, lt):
                    off1 = _slot_off(2 * (lt % 4) + 0)
                    off2 = _slot_off(2 * (lt % 4) + 1)
                    o1 = att_ps[:, off1:off1 + DH2]
                    s1 = att_ps[:, off1 + DH2:off1 + DH2 + 1]
                    o2 = att_ps[:, off2:off2 + DH2]
                    s2 = att_ps[:, off2 + DH2:off2 + DH2 + 1]
                    nc.vector.reciprocal(rec1[u][:, lt:lt + 1], s1)
                    nc.vector.reciprocal(rec2[u][:, lt:lt + 1], s2)
                    # t2 = o2 * (1/s2) * (-lam);  diff = o1*(1/s1) + t2
                    t2 = tpool.tile([P, DH2], F32, tag="t2")
                    nc.vector.tensor_scalar(
                        t2[:], o2, rec2[u][:, lt:lt + 1], lamneg[:],
                        op0=ALU.mult, op1=ALU.mult,
                    )
                    diff = dpool.tile([P, DH2], F16, tag="diff")
                    nc.vector.scalar_tensor_tensor(
                        diff[:], o1, rec1[u][:, lt:lt + 1], t2[:],
                        op0=ALU.mult, op1=ALU.add,
                    )
                    diffs[u][lt] = diff
                    nc.vector.scalar_tensor_tensor(
                        sqjunk[:], diff[:], 1.0, diff[:],
                        op0=ALU.mult, op1=ALU.mult,
                        accum_out=ss_t[u][:, lt:lt + 1],
                    )

                def emit_stats(u, lo, hi):
                    """rsqrt(mean(diff^2)+eps) for lt in [lo, hi) and the
                    final attn2 scale (* rms * (1-lambda_init))."""
                    sl = slice(lo, hi)
                    nc.vector.tensor_scalar(
                        den_t[u][:, sl], ss_t[u][:, sl], 1.0 / DH2, EPS,
                        op0=ALU.mult, op1=ALU.add,
                    )
                    nc.scalar.activation(lnd_t[u][:, sl], den_t[u][:, sl], AF.Ln)
                    nc.scalar.activation(
                        rsc_t[u][:, sl], lnd_t[u][:, sl], AF.Exp, scale=-0.5)
                    for lt in range(lo, hi):
                        nc.vector.tensor_scalar(
                            attn2[:, lt, u, :], diffs[u][lt][:],
                            rsc_t[u][:, lt:lt + 1], 1.0 - LAMBDA_INIT,
                            op0=ALU.mult, op1=ALU.mult,
                        )

                # ---------- lead-in: unit-0 q/k projection + early v ----------
                # order q-nt0, k-nt0 first so the first score MMs (which
                # need only the nt0 halves) can issue as early as possible
                for nt in range(2):
                    for w_t, outt, isq in ((wq_t, qt, True), (wk_t, kt, False)):
                        emit_qk_group(0, w_t, outt, nt, isq)
                for mt in range(3):
                    emit_v_group(mt)

                # ---------- main pipeline ----------
                with tc.tile_pool(name="ps_s", bufs=2, space="PSUM") as ps_s:
                    for u in range(HEFF):
                        for mt in range(NMT):
                            # scoresT MMs: stationary k chunk, moving q;
                            # the two heads run in PE row groups 0-63/64-127
                            # (interleaved h0/h1 so the pairs overlap)
                            pss = [ps_s.tile([P, L], F32, tag="scores",
                                             name=f"ps_{u}_{mt}_{h}")
                                   for h in range(2)]
                            for nl in range(2):
                                for h in range(2):
                                    base = h * HD
                                    nc.tensor.matmul(
                                        pss[h][:, nl * 512:(nl + 1) * 512],
                                        kt[base:base + HD, u, mt * P:(mt + 1) * P],
                                        qt[base:base + HD, u, nl * 512:(nl + 1) * 512],
                                        start=True, stop=True,
                                    )
                            for h in range(2):
                                nc.scalar.activation(
                                    expT[u % 2][:, h, mt, :], pss[h][:], AF.Exp,
                                )
                            # prev unit's attnV rides this slot on the PE
                            if u >= 1:
                                emit_attnv_lt(u - 1, mt)
                                emit_combine_lt(u - 1, mt)
                                if mt == 3:
                                    emit_stats(u - 1, 0, 4)
                                if mt == 7:
                                    emit_stats(u - 1, 4, 8)
                            # remaining projections fill PE slack
                            npop = 2 if u == 0 else 1
                            for _ in range(npop):
                                if projq:
                                    projq.pop()()
                        # end mt loop
                    # end unit loop

                # ---------- tail: attnV(3) + PE transposes + outproj ----------
                with (
                    tc.tile_pool(name="ps_t", bufs=2, space="PSUM") as ps_t,
                    tc.tile_pool(name="ps_o", bufs=2, space="PSUM") as ps_o,
                    tc.tile_pool(name="outsb", bufs=6) as outsb,
                ):
                    u = HEFF - 1

                    def emit_transpose_lt(lt):
                        for uu in range(HEFF):
                            pt = ps_t.tile([P, P], F16, tag="pt")
                            nc.tensor.transpose(
                                pt[:], attn2[:, lt, uu, :], ident[:])
                            if uu % 2 == 0:
                                nc.vector.tensor_copy(attnT[:, lt, uu, :], pt[:])
                            else:
                                nc.scalar.copy(attnT[:, lt, uu, :], pt[:])

                    def emit_outproj_lt(lt):
                        for nt in range(2):
                            ps = ps_o.tile([P, 512], F32, tag="po")
                            for uu in range(HEFF):
                                nc.tensor.matmul(
                                    ps[:],
                                    attnT[:, lt, uu, :],
                                    wo_t[:, uu, nt * 512:(nt + 1) * 512],
                                    start=(uu == 0), stop=(uu == HEFF - 1),
                                )
                            osb = outsb.tile([P, 512], F32, tag="osb")
                            if (lt * 2 + nt) % 2 == 0:
                                nc.vector.tensor_copy(osb[:], ps[:])
                            else:
                                nc.scalar.copy(osb[:], ps[:])
                            nc.gpsimd.dma_start(
                                out_d[lt * P:(lt + 1) * P, nt * 512:(nt + 1) * 512],
                                osb[:],
                            )

                    # per lt-pair: attnV + combine + stats, then transpose
                    # and outproj of the PREVIOUS pair ride the PE between
                    # this pair's matmuls
                    for q in range(4):
                        for lt in (2 * q, 2 * q + 1):
                            emit_attnv_lt(u, lt)
                            emit_combine_lt(u, lt)
                        if q >= 1:
                            emit_transpose_lt(2 * q - 2)
                            emit_transpose_lt(2 * q - 1)
                        emit_stats(u, 2 * q, 2 * q + 2)
                        if q >= 1:
                            emit_outproj_lt(2 * q - 2)
                            emit_outproj_lt(2 * q - 1)
                    for lt2 in (6, 7):
                        emit_transpose_lt(lt2)
                    for lt2 in (6, 7):
                        emit_outproj_lt(lt2)

    if split_waits:
        _split_waits(nc)
    return nc


_NC_CACHE = None


def _get_nc():
    global _NC_CACHE
    if _NC_CACHE is None:
        _NC_CACHE = build_nc(split_waits=(__import__("os").environ.get("NO_SPLIT_WAITS","0")!="1"))
    return _NC_CACHE


def kernel(**inputs):
    nc = _get_nc()
    in_maps = _make_in_maps(inputs)
    res = bass_utils.run_bass_kernel_spmd(nc, in_maps, core_ids=list(range(8)))

    out = np.empty((L, B, D), dtype=np.float32)
    for b in range(B):
        out[:, b, :] = res.results[2 * b]["out"] + res.results[2 * b + 1]["out"]
    return out


def _make_in_maps(inputs):
    query = np.asarray(inputs["query"], dtype=np.float32)
    Wq = np.asarray(inputs["Wq"], dtype=np.float32)
    Wk = np.asarray(inputs["Wk"], dtype=np.float32)
    Wv = np.asarray(inputs["Wv"], dtype=np.float32)
    Wo = np.asarray(inputs["Wo"], dtype=np.float32)
    lq1 = np.asarray(inputs["lq1"], dtype=np.float64)
    lk1 = np.asarray(inputs["lk1"], dtype=np.float64)
    lq2 = np.asarray(inputs["lq2"], dtype=np.float64)
    lk2 = np.asarray(inputs["lk2"], dtype=np.float64)
    lam = float(np.exp(np.sum(lq1 * lk1)) - np.exp(np.sum(lq2 * lk2)) + LAMBDA_INIT)
    lamneg = np.full((P, 1), -lam, dtype=np.float32)
    in_maps = []
    for c in range(8):
        b, hh = c // 2, c % 2
        sl = slice(hh * 512, (hh + 1) * 512)
        in_maps.append({
            "xt": np.ascontiguousarray(query[:, b, :].T.astype(np.float16)),
            "wq": np.ascontiguousarray(Wq[:, sl].astype(np.float16)),
            "wk": np.ascontiguousarray(Wk[:, sl].astype(np.float16)),
            "wv": np.ascontiguousarray(Wv[:, sl].astype(np.float16)),
            "wo": np.ascontiguousarray(Wo[sl, :].astype(np.float16)),
            "lamneg": lamneg,
            "ones": np.ones((P, NMT, HEFF, 1), dtype=np.float16),
            "ident": np.eye(P, dtype=np.float16),
        })
    return in_maps


def kernel_traced(**inputs):
    """Run with NTFF tracing; returns max-core exec time in ns (or None)."""
    nc = _get_nc()
    res = bass_utils.run_bass_kernel_spmd(
        nc, _make_in_maps(inputs), core_ids=list(range(8)), trace=True,
    )
    if res.instructions_and_trace is not None:
        print("trace:", res.instructions_and_trace[1])
    print("per-core mean exec:", res.mean_exec_time_ns,
          "max core:", res.max_exec_time_core_id)
    return res.exec_time_ns


# revision 33
# speedup vs baseline: 1.0573x; 1.0292x over previous
"""Differential multi-head attention on 8 TRN2 NeuronCores.

Sharding: core c handles batch b = c//2 and head-half hh = c%2
(4 of 8 effective heads = 8 of 16 raw heads). Host sums the two
per-batch partial out-projections (the "all-reduce") and reassembles
(L, N, D) fp32.

Per-core scheme (v2, transposed scores):
  scoresT[m, l] = k_m . q_l computed directly on the PE (stationary =
  k chunk, moving = q), so exp(scoresT) tiles are exactly the
  stationary operand attnV needs -- no SBUF transposes of the
  probability matrix and no full-matrix lambda-combine on DVE.
  Softmax row sums ride along as a ones-column appended to V: each
  attnV matmul outputs 129 cols = [attn@V | rowsum]. The differential
  combine o1/s1 - lam*o2/s2 and headwise RMSNorm run on small
  [128,128] tiles. Unit u's scores+exp (ACT-bound) overlap unit u-1's
  attnV matmuls (PE), with q/k/v projections filling remaining PE
  slack. rsqrt for RMSNorm is exp(-0.5*ln(x)) to stay in the one ACT
  table set (natural_log_exp).

All matmuls in fp16 with fp32 PSUM accumulation.
"""
import os

import numpy as np

import concourse.bass as bass
import concourse.mybir as mybir
import concourse.tile as tile
from concourse import bass_utils


def _enable_fwl():
    """Re-enable the compiler's fast-weight-load path (off by default in
    this environment); stationary-operand loads then run at 2 cols/cycle."""
    try:
        from concourse import compiler_utils

        flags = compiler_utils.get_compiler_flags()
        new = [f.replace("--enable-ldw-opt=false", "--enable-ldw-opt=true")
               for f in flags]
        if new != flags:
            compiler_utils.set_compiler_flags(new)
    except Exception:
        pass


if os.environ.get("BASS_FWL", "0") == "1":
    _enable_fwl()

L = 1024          # sequence length
B = 4             # batch
D = 1024          # embed dim
P = 128           # partitions
HD = 64           # head dim
HEFF = 4          # effective heads per core (of 8 total)
DH2 = 2 * HD      # 128, v head dim / rmsnorm width
KO = D // P       # 8 contraction chunks
NMT = L // P      # 8 m-chunks (key/value tiles)
NLT = L // P      # 8 l-tiles (query tiles)
LAMBDA_INIT = 0.8
EPS = 1e-5
SCALING = HD ** -0.5
VW = 132          # vt row width: 128 v cols + ones col + pad

F32 = mybir.dt.float32
F16 = mybir.dt.float16
AF = mybir.ActivationFunctionType
ALU = mybir.AluOpType

# ---------------------------------------------------------------------------
# wait-budget post-pass (TRN2 ISA instructions carry a single wait slot;
# excess waits move to InstNoOp on the same engine stream)
_WAIT_EXEMPT = {
    "InstEventSemaphore", "InstRegisterMove", "InstUnconditionalBranch",
    "InstCall", "InstHalt", "InstNoOp", "InstAllEngineBarrier",
    "InstBranchHint", "InstCompareAndBranch", "InstFusedRegOps",
    "InstRegisterAlu",
}
_waitfix_counter = [0]


def _split_waits(nc):
    n_split = 0
    for f in nc.m.functions:
        for bb in f.blocks:
            il = bb.instructions
            out = []
            changed = False
            for inst in il:
                tn = type(inst).__name__
                si = inst.sync_info
                waits = list(si.on_wait) if si is not None and si.on_wait else []
                if tn in _WAIT_EXEMPT or len(waits) <= 1:
                    out.append(inst)
                    continue
                excess, keep = waits[:-1], waits[-1:]
                movable = [w for w in excess if w.wait_reg is None]
                stuck = [w for w in excess if w.wait_reg is not None]
                for w in movable:
                    _waitfix_counter[0] += 1
                    out.append(mybir.InstNoOp(
                        name=f"I-waitnop-{_waitfix_counter[0]}",
                        engine=inst.engine, ins=[], outs=[],
                        sync_info=mybir.SyncInfo(on_wait=[w], on_update=[]),
                    ))
                    n_split += 1
                si.on_wait = stuck + keep
                changed = True
                out.append(inst)
            if changed:
                bb.instructions = out
    return n_split


# psum slot offsets for the attnV accumulators: 8 slots of 129 fp32 at a
# 132-word (528B, 16B-aligned) stride, packed 3-3-2 into 3 banks (512 fp32
# each) so no slot crosses a bank
def _slot_off(i):
    return (i // 3) * 512 + (i % 3) * 132


# ---------------------------------------------------------------------------

def build_nc(split_waits=True):
    nc = bass.Bass("TRN2", target_bir_lowering=False, debug=False)

    xt_d = nc.dram_tensor("xt", [D, L], F16, kind="ExternalInput").ap()
    wq_d = nc.dram_tensor("wq", [D, HEFF * DH2], F16, kind="ExternalInput").ap()
    wk_d = nc.dram_tensor("wk", [D, HEFF * DH2], F16, kind="ExternalInput").ap()
    wv_d = nc.dram_tensor("wv", [D, HEFF * DH2], F16, kind="ExternalInput").ap()
    wo_d = nc.dram_tensor("wo", [HEFF * DH2, D], F16, kind="ExternalInput").ap()
    lam_d = nc.dram_tensor("lamneg", [P, 1], F32, kind="ExternalInput").ap()
    ones_d = nc.dram_tensor("ones", [P, NMT, HEFF, 1], F16,
                            kind="ExternalInput").ap()
    ident_d = nc.dram_tensor("ident", [P, P], F16, kind="ExternalInput").ap()
    out_d = nc.dram_tensor("out", [L, D], F32, kind="ExternalOutput").ap()

    with tile.TileContext(nc) as tc:
        with (
            tc.tile_pool(name="weights", bufs=1) as wpool,
            tc.tile_pool(name="proj", bufs=1) as projpool,
            tc.tile_pool(name="stats", bufs=1) as spool,
            tc.tile_pool(name="attn", bufs=1) as apool,
        ):
            # ---------------- loads ----------------
            lamneg = wpool.tile([P, 1], F32)
            nc.gpsimd.dma_start(lamneg[:], lam_d[:])
            xt_t = wpool.tile([P, KO, L], F16)
            xt_r = xt_d.rearrange("(ko p) l -> p ko l", p=P)
            wq_t = wpool.tile([P, KO, 512], F16)
            wq_r = wq_d.rearrange("(ko p) n -> p ko n", p=P)
            wk_t = wpool.tile([P, KO, 512], F16)
            wk_r = wk_d.rearrange("(ko p) n -> p ko n", p=P)
            wv_t = wpool.tile([P, KO, 512], F16)
            wv_r = wv_d.rearrange("(ko p) n -> p ko n", p=P)
            # sync queue: first x/wq chunks then all of wv (so v-proj can
            # start during the lead); gpsimd queue: the rest
            nc.sync.dma_start(wq_t[:, 0], wq_r[:, 0])
            nc.sync.dma_start(xt_t[:, 0], xt_r[:, 0])
            for ko in range(1, KO):
                nc.gpsimd.dma_start(wq_t[:, ko], wq_r[:, ko])
                nc.gpsimd.dma_start(xt_t[:, ko], xt_r[:, ko])
            for ko in range(KO):
                nc.sync.dma_start(wv_t[:, ko], wv_r[:, ko])
                nc.gpsimd.dma_start(wk_t[:, ko], wk_r[:, ko])
            wo_t = wpool.tile([P, HEFF, D], F16)
            nc.gpsimd.dma_start(wo_t[:], wo_d.rearrange("(u p) n -> p u n", p=P))

            act_scr = spool.tile([P, 4], F32)
            dve_scr = spool.tile([P, 4], F32)
            # init touches: load the exp/ln ACT table set early (overlaps
            # the input DMAs) and let DVE observe the consts load
            nc.vector.tensor_copy(dve_scr[0:1, 0:1], lamneg[0:1, 0:1])
            nc.scalar.activation(act_scr[0:1, 0:1], lamneg[0:1, 0:1], AF.Exp)
            nc.scalar.activation(act_scr[0:1, 1:2], act_scr[0:1, 0:1], AF.Ln)

            # ---------------- persistent SBUF tensors ----------------
            qt = projpool.tile([P, HEFF, L], F16)    # (dh%128, u, l)
            kt = projpool.tile([P, HEFF, L], F16)
            vt = projpool.tile([P, NMT, HEFF, VW], F16)  # (m%128, mt, u, d|1)
            nc.gpsimd.dma_start(vt[:, :, :, DH2:DH2 + 1], ones_d[:])
            ident = projpool.tile([P, P], F16)
            nc.gpsimd.dma_start(ident[:], ident_d[:])

            # exp(scoresT) double buffer: [m%128, h, mt, l] per unit parity
            expT = [apool.tile([P, 2, NMT, L], F16, name=f"expT{i}")
                    for i in range(2)]
            attn2 = apool.tile([P, NLT, HEFF, DH2], F16)  # (l%128, lt, u, d)
            attnT = apool.tile([P, NLT, HEFF, P], F16)    # (d, lt, u, l%128)

            rec1 = [spool.tile([P, NLT], F32, name=f"rec1_{u}") for u in range(HEFF)]
            rec2 = [spool.tile([P, NLT], F32, name=f"rec2_{u}") for u in range(HEFF)]
            ss_t = [spool.tile([P, NLT], F32, name=f"ss_{u}") for u in range(HEFF)]
            den_t = [spool.tile([P, NLT], F32, name=f"den_{u}") for u in range(HEFF)]
            lnd_t = [spool.tile([P, NLT], F32, name=f"lnd_{u}") for u in range(HEFF)]
            rsc_t = [spool.tile([P, NLT], F32, name=f"rsc_{u}") for u in range(HEFF)]
            sqjunk = spool.tile([P, DH2], F16)

            with (
                tc.tile_pool(name="diffs", bufs=10) as dpool,
                tc.tile_pool(name="t2s", bufs=4) as tpool,
                tc.tile_pool(name="outsb", bufs=6) as outsb,
            ):
                # PSUM bank plan (manual alloc/release; left and right
                # sides are independent LIFO stacks):
                #   right: scores 2x[128,1024] (4 banks) until last exp,
                #          then ps_t(2) + ps_o(2) for the tail
                #   left:  proj 2x[128,512] during lead+unit0, then
                #          attnV accumulators (3 banks) for units 1..3+tail
                ps_s = tc.alloc_tile_pool(
                    name="ps_s", bufs=2, space="PSUM", side="right")
                ps_proj = tc.alloc_tile_pool(
                    name="ps_proj", bufs=2, space="PSUM", side="left")
                att_ps = None  # allocated after proj releases
                ps_t = ps_o = None

                # ---------- projection group emitters ----------
                def emit_qk_group(u, w_t, outt, nt, isq):
                    ps = ps_proj.tile([P, 512], F32, tag="pp")
                    for ko in range(KO):
                        nc.tensor.matmul(
                            ps[:],
                            w_t[:, ko, u * P:(u + 1) * P],
                            xt_t[:, ko, nt * 512:(nt + 1) * 512],
                            start=(ko == 0), stop=(ko == KO - 1),
                        )
                    dst = outt[:, u, nt * 512:(nt + 1) * 512]
                    if isq:
                        nc.vector.tensor_scalar_mul(dst, ps[:], SCALING)
                    else:
                        nc.vector.tensor_copy(dst, ps[:])

                def emit_v_group(mt):
                    ps = ps_proj.tile([P, 512], F32, tag="pp")
                    for ko in range(KO):
                        nc.tensor.matmul(
                            ps[:],
                            xt_t[:, ko, mt * P:(mt + 1) * P],
                            wv_t[:, ko, :],
                            start=(ko == 0), stop=(ko == KO - 1),
                        )
                    nc.vector.tensor_copy(
                        vt[:, mt, :, 0:DH2],
                        ps[:].rearrange("p (u d) -> p u d", u=HEFF),
                    )

                # proj work queue, all consumed during unit-0 slots: v
                # groups first (attnV(0) needs all of v at unit-1 start),
                # then q before k per unit (scores need q fully, k
                # per-chunk)
                projq = []
                for mt in range(NMT):
                    projq.append(lambda mt=mt: emit_v_group(mt))
                for u in range(1, HEFF):
                    for w_t, outt, isq in ((wq_t, qt, True), (wk_t, kt, False)):
                        for nt in range(2):
                            projq.append(
                                lambda u=u, w=w_t, o=outt, nt=nt, q=isq:
                                    emit_qk_group(u, w, o, nt, q))
                projq.reverse()  # pop from end
                npop0 = [3, 3, 3, 3, 2, 2, 2, 2]  # per unit-0 slot

                # ---------- attnV + combine emitters ----------
                diffs = [[None] * NLT for _ in range(HEFF)]

                def emit_attnv_lt(u, lt):
                    """attnV for (u, lt), both heads; 16 MMs + combine."""
                    par = u % 2
                    for h in range(2):
                        off = _slot_off(2 * (lt % 4) + h)
                        pav = att_ps[:, off:off + 129]
                        for mt in range(NMT):
                            nc.tensor.matmul(
                                pav,
                                expT[par][:, h, mt, lt * P:(lt + 1) * P],
                                vt[:, mt, u, 0:DH2 + 1],
                                start=(mt == 0), stop=(mt == NMT - 1),
                            )

                def emit_combine_lt(u, lt):
                    off1 = _slot_off(2 * (lt % 4) + 0)
                    off2 = _slot_off(2 * (lt % 4) + 1)
                    o1 = att_ps[:, off1:off1 + DH2]
                    s1 = att_ps[:, off1 + DH2:off1 + DH2 + 1]
                    o2 = att_ps[:, off2:off2 + DH2]
                    s2 = att_ps[:, off2 + DH2:off2 + DH2 + 1]
                    nc.vector.reciprocal(rec1[u][:, lt:lt + 1], s1)
                    nc.vector.reciprocal(rec2[u][:, lt:lt + 1], s2)
                    # t2 = o2 * (1/s2) * (-lam);  diff = o1*(1/s1) + t2
                    t2 = tpool.tile([P, DH2], F32, tag="t2")
                    nc.vector.tensor_scalar(
                        t2[:], o2, rec2[u][:, lt:lt + 1], lamneg[:],
                        op0=ALU.mult, op1=ALU.mult,
                    )
                    diff = dpool.tile([P, DH2], F16, tag="diff")
                    nc.vector.scalar_tensor_tensor(
                        diff[:], o1, rec1[u][:, lt:lt + 1], t2[:],
                        op0=ALU.mult, op1=ALU.add,
                    )
                    diffs[u][lt] = diff
                    nc.vector.scalar_tensor_tensor(
                        sqjunk[:], diff[:], 1.0, diff[:],
                        op0=ALU.mult, op1=ALU.mult,
                        accum_out=ss_t[u][:, lt:lt + 1],
                    )

                def emit_stats(u, lo, hi):
                    """rsqrt(mean(diff^2)+eps) for lt in [lo, hi) and the
                    final attn2 scale (* rms * (1-lambda_init))."""
                    sl = slice(lo, hi)
                    nc.vector.tensor_scalar(
                        den_t[u][:, sl], ss_t[u][:, sl], 1.0 / DH2, EPS,
                        op0=ALU.mult, op1=ALU.add,
                    )
                    nc.scalar.activation(lnd_t[u][:, sl], den_t[u][:, sl], AF.Ln)
                    nc.scalar.activation(
                        rsc_t[u][:, sl], lnd_t[u][:, sl], AF.Exp, scale=-0.5)
                    for lt in range(lo, hi):
                        nc.vector.tensor_scalar(
                            attn2[:, lt, u, :], diffs[u][lt][:],
                            rsc_t[u][:, lt:lt + 1], 1.0 - LAMBDA_INIT,
                            op0=ALU.mult, op1=ALU.mult,
                        )

                def emit_transpose_lt(lt):
                    for uu in range(HEFF):
                        pt = ps_t.tile([P, P], F16, tag="pt")
                        nc.tensor.transpose(
                            pt[:], attn2[:, lt, uu, :], ident[:])
                        if uu % 2 == 0:
                            nc.vector.tensor_copy(attnT[:, lt, uu, :], pt[:])
                        else:
                            nc.scalar.copy(attnT[:, lt, uu, :], pt[:])

                def emit_outproj_lt(lt):
                    for nt in range(2):
                        ps = ps_o.tile([P, 512], F32, tag="po")
                        for uu in range(HEFF):
                            nc.tensor.matmul(
                                ps[:],
                                attnT[:, lt, uu, :],
                                wo_t[:, uu, nt * 512:(nt + 1) * 512],
                                start=(uu == 0), stop=(uu == HEFF - 1),
                            )
                        osb = outsb.tile([P, 512], F32, tag="osb")
                        if (lt * 2 + nt) % 2 == 0:
                            nc.vector.tensor_copy(osb[:], ps[:])
                        else:
                            nc.scalar.copy(osb[:], ps[:])
                        nc.gpsimd.dma_start(
                            out_d[lt * P:(lt + 1) * P, nt * 512:(nt + 1) * 512],
                            osb[:],
                        )

                def emit_scores_slot(u, mt):
                    # scoresT MMs: stationary k chunk, moving q; the two
                    # heads run in PE row groups 0-63/64-127 (interleaved
                    # h0/h1 so the pairs overlap)
                    pss = [ps_s.tile([P, L], F32, tag="scores",
                                     name=f"ps_{u}_{mt}_{h}")
                           for h in range(2)]
                    for nl in range(2):
                        for h in range(2):
                            base = h * HD
                            nc.tensor.matmul(
                                pss[h][:, nl * 512:(nl + 1) * 512],
                                kt[base:base + HD, u, mt * P:(mt + 1) * P],
                                qt[base:base + HD, u, nl * 512:(nl + 1) * 512],
                                start=True, stop=True,
                            )
                    for h in range(2):
                        nc.scalar.activation(
                            expT[u % 2][:, h, mt, :], pss[h][:], AF.Exp,
                        )

                # ---------- lead-in: unit-0 q/k projection ----------
                # order q-nt0, k-nt0 first so the first score MMs (which
                # need only the nt0 halves) can issue as early as possible
                for nt in range(2):
                    for w_t, outt, isq in ((wq_t, qt, True), (wk_t, kt, False)):
                        emit_qk_group(0, w_t, outt, nt, isq)

                # ---------- unit 0: scores/exp + all remaining proj ----------
                for mt in range(NMT):
                    emit_scores_slot(0, mt)
                    for _ in range(npop0[mt]):
                        if projq:
                            projq.pop()()
                assert not projq
                ps_proj.release()
                ps_att = tc.alloc_tile_pool(
                    name="ps_att", bufs=1, space="PSUM", side="left")
                att_ps = ps_att.tile([P, 3 * 512], F32)  # 3 banks, 8 slots

                # ---------- units 1..3: scores/exp + prev unit attnV ----------
                for u in range(1, HEFF):
                    for mt in range(NMT):
                        emit_scores_slot(u, mt)
                        emit_attnv_lt(u - 1, mt)
                        emit_combine_lt(u - 1, mt)
                        if mt == 3:
                            emit_stats(u - 1, 0, 4)
                        if mt == 7:
                            emit_stats(u - 1, 4, 8)

                # ---------- tail: attnV(3) + PE transposes + outproj ----------
                ps_s.release()
                ps_t = tc.alloc_tile_pool(
                    name="ps_t", bufs=2, space="PSUM", side="right")
                ps_o = tc.alloc_tile_pool(
                    name="ps_o", bufs=2, space="PSUM", side="right")
                u = HEFF - 1
                # per lt-pair: attnV + combine + stats, then transpose and
                # outproj of the PREVIOUS pair ride the PE between this
                # pair's matmuls
                for q in range(4):
                    for lt in (2 * q, 2 * q + 1):
                        emit_attnv_lt(u, lt)
                        emit_combine_lt(u, lt)
                    if q >= 1:
                        emit_transpose_lt(2 * q - 2)
                        emit_transpose_lt(2 * q - 1)
                    emit_stats(u, 2 * q, 2 * q + 2)
                    if q >= 1:
                        emit_outproj_lt(2 * q - 2)
                        emit_outproj_lt(2 * q - 1)
                for lt2 in (6, 7):
                    emit_transpose_lt(lt2)
                for lt2 in (6, 7):
                    emit_outproj_lt(lt2)
                ps_o.release()
                ps_t.release()
                ps_att.release()

    if split_waits:
        _split_waits(nc)
    return nc


_NC_CACHE = None


def _get_nc():
    global _NC_CACHE
    if _NC_CACHE is None:
        _NC_CACHE = build_nc(split_waits=(__import__("os").environ.get("NO_SPLIT_WAITS","0")!="1"))
    return _NC_CACHE


def kernel(**inputs):
    nc = _get_nc()
    in_maps = _make_in_maps(inputs)
    res = bass_utils.run_bass_kernel_spmd(nc, in_maps, core_ids=list(range(8)))

    out = np.empty((L, B, D), dtype=np.float32)
    for b in range(B):
        out[:, b, :] = res.results[2 * b]["out"] + res.results[2 * b + 1]["out"]
    return out


def _make_in_maps(inputs):
    query = np.asarray(inputs["query"], dtype=np.float32)
    Wq = np.asarray(inputs["Wq"], dtype=np.float32)
    Wk = np.asarray(inputs["Wk"], dtype=np.float32)
    Wv = np.asarray(inputs["Wv"], dtype=np.float32)
    Wo = np.asarray(inputs["Wo"], dtype=np.float32)
    lq1 = np.asarray(inputs["lq1"], dtype=np.float64)
    lk1 = np.asarray(inputs["lk1"], dtype=np.float64)
    lq2 = np.asarray(inputs["lq2"], dtype=np.float64)
    lk2 = np.asarray(inputs["lk2"], dtype=np.float64)
    lam = float(np.exp(np.sum(lq1 * lk1)) - np.exp(np.sum(lq2 * lk2)) + LAMBDA_INIT)
    lamneg = np.full((P, 1), -lam, dtype=np.float32)
    in_maps = []
    for c in range(8):
        b, hh = c // 2, c % 2
        sl = slice(hh * 512, (hh + 1) * 512)
        in_maps.append({
            "xt": np.ascontiguousarray(query[:, b, :].T.astype(np.float16)),
            "wq": np.ascontiguousarray(Wq[:, sl].astype(np.float16)),
            "wk": np.ascontiguousarray(Wk[:, sl].astype(np.float16)),
            "wv": np.ascontiguousarray(Wv[:, sl].astype(np.float16)),
            "wo": np.ascontiguousarray(Wo[sl, :].astype(np.float16)),
            "lamneg": lamneg,
            "ones": np.ones((P, NMT, HEFF, 1), dtype=np.float16),
            "ident": np.eye(P, dtype=np.float16),
        })
    return in_maps


def kernel_traced(**inputs):
    """Run with NTFF tracing; returns max-core exec time in ns (or None)."""
    nc = _get_nc()
    res = bass_utils.run_bass_kernel_spmd(
        nc, _make_in_maps(inputs), core_ids=list(range(8)), trace=True,
    )
    if res.instructions_and_trace is not None:
        print("trace:", res.instructions_and_trace[1])
    print("per-core mean exec:", res.mean_exec_time_ns,
          "max core:", res.max_exec_time_core_id)
    return res.exec_time_ns
